# revision 2
# baseline (speedup 1.0000x reference)
"""LoRA linear kernel for Trainium2 (Bass/Tile), 8-core SPMD — v4.

out = x @ (A @ B), reassociated as (x @ A) @ B, data-parallel over rows
of x (2048 rows per core), all matmul operands bf16, bf16 output
(upcast on host).  x is pre-transposed per shard on the host.

v4: two-half m-pipeline (1024 rows per half, 2KB DMA rows) so half 1's
input DMA overlaps half 0's mm2 + output DMA.  Output DMAs are issued
from the gpsimd queue (qGpSimdDynamic) so they can't head-of-line block
input DMAs on the sync queue (qSyncDynamicHW).
"""

import os
import sys

import numpy as np

for _p in ("/opt/trn_rl_repo",):
    if os.path.isdir(_p) and _p not in sys.path:
        sys.path.insert(0, _p)

import concourse.bacc as bacc
import concourse.mybir as mybir
from concourse import tile
from concourse.bass_utils import run_bass_kernel_spmd

import ml_dtypes

R = 16
B_DIM = 4
SEQ = 4096
K = 4096  # in_features
N = 4096  # out_features
M_FULL = B_DIM * SEQ  # 16384
NCORES = 8
M_SHARD = M_FULL // NCORES  # 2048
SCALING = 16.0 / 16.0  # alpha / r == 1.0

KC = 128  # contraction chunk (partition dim of mm1)
N_KC = K // KC  # 32
MT = 128  # rows per m-tile (partition dim of mm2 output)
NCH = 512  # matmul2 output chunk (one PSUM bank of fp32)
N_NCH = N // NCH  # 8
MH = 1024  # m-half size
N_MH = M_SHARD // MH  # 2
MSUB = 512  # mm1 psum tile free size
N_MSUB = MH // MSUB  # 2
MT_PER_H = MH // MT  # 8

_F32 = mybir.dt.float32
_BF16 = mybir.dt.bfloat16


def _build_kernel(tc, nc, xT, a_pre, b_in, out):
    with (
        tc.tile_pool(name="const", bufs=1) as cpool,
        tc.tile_pool(name="xin", bufs=10) as xpool,
        tc.tile_pool(name="tps", bufs=2, space="PSUM") as tpsum,
        tc.tile_pool(name="tsb", bufs=2) as tspool,
        tc.tile_pool(name="ops", bufs=4, space="PSUM") as opsum,
        tc.tile_pool(name="osb", bufs=4) as opool,
    ):
        a_sb = cpool.tile([128, N_KC * R], _BF16, name="a_sb")
        nc.sync.dma_start(out=a_sb, in_=a_pre)
        b_sb = cpool.tile([R, N], _BF16, name="b_sb")
        nc.sync.dma_start(out=b_sb, in_=b_in)

        ci = 0
        for h in range(N_MH):
            m0 = h * MH
            # ---- mm1 for this half: tT[r, m0:m0+1024] ----
            tps = [
                tpsum.tile([R, MSUB], _F32, name=f"tps{j}", bufs=1)
                for j in range(N_MSUB)
            ]
            for c in range(N_KC):
                xc = xpool.tile([128, MH], _BF16, name="xc")
                nc.sync.dma_start(
                    out=xc, in_=xT[c * KC : (c + 1) * KC, m0 : m0 + MH]
                )
                for j in range(N_MSUB):
                    nc.tensor.matmul(
                        tps[j][:],
                        a_sb[:, c * R : (c + 1) * R],
                        xc[:, j * MSUB : (j + 1) * MSUB],
                        start=(c == 0),
                        stop=(c == N_KC - 1),
                    )

            tsb = tspool.tile([R, MH], _BF16, name="tsb")
            for j in range(N_MSUB):
                nc.vector.tensor_copy(tsb[:, j * MSUB : (j + 1) * MSUB], tps[j][:])

            # ---- mm2 for this half's 8 m-tiles ----
            for mt in range(MT_PER_H):
                osb = opool.tile([MT, N], _BF16, name="osb")
                for j in range(N_NCH):
                    ops = opsum.tile([MT, NCH], _F32, name="ops")
                    nc.tensor.matmul(
                        ops[:],
                        tsb[:, mt * MT : (mt + 1) * MT],
                        b_sb[:, j * NCH : (j + 1) * NCH],
                        start=True,
                        stop=True,
                    )
                    dst = osb[:, j * NCH : (j + 1) * NCH]
                    # Split PSUM->SBUF downcast copies ~5:3 DVE:ACT.
                    if ci % 8 < 5:
                        nc.vector.tensor_copy(dst, ops[:])
                    else:
                        nc.scalar.copy(dst, ops[:])
                    ci += 1
                row0 = m0 + mt * MT
                nc.gpsimd.dma_start(out=out[row0 : row0 + MT, :], in_=osb[:])


_NC_CACHE = None


def _get_nc():
    global _NC_CACHE
    if _NC_CACHE is not None:
        return _NC_CACHE
    nc = bacc.Bacc("TRN2", target_bir_lowering=False, debug=False)
    xT = nc.dram_tensor("xT", [K, M_SHARD], _BF16, kind="ExternalInput").ap()
    a_pre = nc.dram_tensor("a_pre", [128, N_KC * R], _BF16, kind="ExternalInput").ap()
    b_in = nc.dram_tensor("b_in", [R, N], _BF16, kind="ExternalInput").ap()
    out = nc.dram_tensor("out", [M_SHARD, N], _BF16, kind="ExternalOutput").ap()
    with tile.TileContext(nc) as tc:
        _build_kernel(tc, nc, xT, a_pre, b_in, out)
    nc.compile()
    _NC_CACHE = nc
    return nc


LAST_RESULTS = None


def kernel(x: np.ndarray, A: np.ndarray, B: np.ndarray) -> np.ndarray:
    global LAST_RESULTS
    assert x.shape == (B_DIM, SEQ, K), x.shape
    assert A.shape == (K, R), A.shape
    assert B.shape == (R, N), B.shape

    x_np = np.asarray(x, dtype=np.float32).reshape(M_FULL, K)
    a_np = np.asarray(A, dtype=np.float32)
    b_np = (np.asarray(B, dtype=np.float32) * SCALING).astype(ml_dtypes.bfloat16)

    a_pre = np.ascontiguousarray(
        a_np.reshape(N_KC, KC, R).transpose(1, 0, 2).reshape(128, N_KC * R)
    ).astype(ml_dtypes.bfloat16)

    in_maps = []
    for i in range(NCORES):
        shard = x_np[i * M_SHARD : (i + 1) * M_SHARD]
        xt = np.ascontiguousarray(shard.astype(ml_dtypes.bfloat16).T)
        in_maps.append({"xT": xt, "a_pre": a_pre, "b_in": b_np})

    nc = _get_nc()
    trace = os.environ.get("KERNEL_TRACE", "0") == "1"
    tmpdir = os.environ.get("KERNEL_TMPDIR") or None
    res = run_bass_kernel_spmd(
        nc, in_maps, core_ids=list(range(NCORES)), trace=trace, tmpdir=tmpdir
    )
    LAST_RESULTS = res
    out = np.concatenate(
        [np.asarray(res.results[i]["out"]) for i in range(NCORES)], axis=0
    ).astype(np.float32)
    return out.reshape(B_DIM, SEQ, N)
